# revision 44
# baseline (speedup 1.0000x reference)
"""Causal self-attention with RoPE — Trainium2 Bass/Tile kernel.

Problem: B=2, T=2048, C=2048, H=16 heads, D=128 head dim.
    qkv = x @ w_qkv ; RoPE(q, k) ; causal softmax attention ; out = attn_out @ w_out

Sharding (8 cores): core c handles batch b = c//4 and the 4 heads
hg = c%4 (heads 4*hg .. 4*hg+3).  Each core computes
    partial_c = attn_bh(x[b]) @ w_out[rows of its heads]      (shape [T, C])
(bf16) and the host all-reduces in fp32: out[b] = sum of the 4 partials.

Per-core pipeline (all matmuls bf16 inputs, fp32 PSUM accumulate):
  A) Fused QKV projection from ONE pass over x (one batched DMA per
     512-token chunk).  q,k produced transposed ([D, T], feature-major) so
     scores/attn matmuls need no transposes; v produced natural ([T, D]).
     RoPE applied in [D, T] layout.
  B) Flash-style causal attention, one flattened software-pipelined stream
     of key-tile pairs across all (head, 512-query chunk) iterations:
     scores for pair n+1 issue before attn@v of pair n, so TensorE never
     waits on ScalarE's exp.  Causal mask ADDED in PSUM via a
     maskT@identity matmul (-3e4 above the diagonal -> exp==0 exactly);
     diagonal key tiles narrowed to causally-live columns everywhere.
     Softmax denominators: off-diagonal exp pairs are folded on the (idle)
     GPSIMD engine, one all-ones [128,128] matmul per fold (lagged one
     pair) accumulates broadcast row-sums in PSUM; 1/rowsum on DVE (bf16)
     + multiply -> normalized outT (bf16).
  D) Out-projection -> partial [T, C] bf16; PSUM->SBUF copies alternate
     ScalarE/DVE; one DMA per 4 row-tiles.
"""

import sys

for _p in ("/opt/trn_rl_repo",):
    if _p not in sys.path:
        sys.path.insert(0, _p)

import numpy as np
import ml_dtypes

import concourse.bass as bass
import concourse.mybir as mybir
import concourse.tile as tile

BF = mybir.dt.bfloat16
FP = mybir.dt.float32

BF_NP = ml_dtypes.bfloat16

NUM_HEADS = 16
B, T_FULL, C_FULL = 2, 2048, 2048
D = 128
N_CORES = 8
HPC = 4  # heads per core

ROPE_THETA = 10000.0
MASK_NEG = -30000.0  # additive causal mask; exp(scale * -3e4) == 0 exactly


def _split_multi_waits(nc):
    """This container's walrus supports only ONE sync-wait per instruction
    ("Too many sync wait commands").  Hoist all but one wait of every
    multi-wait instruction onto preceding EventSemaphore instructions
    executed by the same engine's sequencer (block order = program order per
    engine) — same semantics, codegen-legal."""
    import bass_rust

    skip = (mybir.InstEventSemaphore,)
    ctr = 0
    for fn in nc.m.functions:
        for blk in fn.blocks:
            new_insts = None
            for idx, inst in enumerate(blk.instructions):
                si = inst.sync_info
                if (
                    not isinstance(inst, skip)
                    and si is not None
                    and si.on_wait
                    and len(si.on_wait) > 1
                ):
                    if new_insts is None:
                        new_insts = list(blk.instructions[:idx])
                    # keep the first wait (the data-dep one, usually latest to
                    # resolve) on the instruction itself; hoist the rest.
                    for w in si.on_wait[1:]:
                        ev = mybir.InstEventSemaphore(
                            name=f"I-dmaw{ctr}", ins=[], outs=[]
                        )
                        ctr += 1
                        ev.sync_info = bass_rust.SyncInfo(
                            on_wait=[w], on_update=[]
                        )
                        ev.engine = inst.engine
                        new_insts.append(ev)
                    inst.sync_info = bass_rust.SyncInfo(
                        on_wait=[si.on_wait[0]], on_update=si.on_update or []
                    )
                    new_insts.append(inst)
                elif new_insts is not None:
                    new_insts.append(inst)
            if new_insts is not None:
                blk.instructions = new_insts


class Cfg:
    """Kernel geometry. Full-size by default; shrinkable for simulator tests."""

    def __init__(self, T=T_FULL, C=C_FULL, hpc=HPC):
        assert T % 512 == 0 and C % 128 == 0
        self.T = T
        self.C = C
        self.hpc = hpc
        self.scale = 1.0 / np.sqrt(D)
        self.c_tiles = C // 128      # contraction tiles for QKV
        self.t_chunks = T // 512     # token chunks (QKV + queries)
        self.t_tiles = T // 128      # token tiles (keys / out rows)
        self.n_chunks = C // 512     # output-feature chunks for out-proj


def build_attention(cfg: Cfg):
    """Build the SPMD Bass program (identical on all cores; data differs)."""
    nc = bass.Bass("TRN2", debug=False, enable_partition_id=False)
    T, C, hpc = cfg.T, cfg.C, cfg.hpc
    F = hpc * D  # per-core q (or k, or v) feature count
    nct = cfg.c_tiles

    xT = nc.dram_tensor("xT", [C, T], BF, kind="ExternalInput")
    # wqk pre-packed per output-feature tile: [ft, p, (cc f)] so one 2D DMA
    # fetches one ft's full [C-chunk=128, C] weight tile.
    wqk = nc.dram_tensor("wqk", [2 * hpc * 128, C], BF, kind="ExternalInput")
    wv = nc.dram_tensor("wv", [C, F], BF, kind="ExternalInput")
    wout = nc.dram_tensor("wout", [F, C], BF, kind="ExternalInput")
    # cos|sin [D, 2T] (sin sign-baked); ones|ident|maskT packed [128, 384]
    cossinT = nc.dram_tensor("cossinT", [D, 2 * T], BF, kind="ExternalInput")
    kconsts = nc.dram_tensor("kconsts", [128, 384], BF, kind="ExternalInput")
    out = nc.dram_tensor("out", [T, C], BF, kind="ExternalOutput")

    Exp = mybir.ActivationFunctionType.Exp

    with tile.TileContext(nc) as tc:
        with (
            tc.tile_pool(name="consts", bufs=1) as consts,
            tc.tile_pool(name="persist", bufs=1) as persist,
            tc.tile_pool(name="otp", bufs=1) as otp,
            tc.tile_pool(name="wo_pool", bufs=1) as wo_pool,
            tc.tile_pool(name="wqk_pool", bufs=1) as wqk_pool,
            tc.tile_pool(name="wv_pool", bufs=1) as wv_pool,
        ):
            # --- staging: few, large DMAs (each ~614ns of queue time) ---
            # ACT ring: wqk ft0, x0-half, wqk ft1..7, consts
            # SP  ring: x0-half, x1, wv, x2, x3 (prefetched in-loop), outs
            wqkf_sb = [
                wqk_pool.tile([128, C], BF, name=f"wqkf_sb{ft}", tag=f"wqk{ft}")
                for ft in range(2 * hpc)
            ]

            def wqkf(ft):
                return wqkf_sb[ft]

            wv_sb = wv_pool.tile([128, nct, F], BF, name="wv_sb")

            # q/k transposed [D, T] per head (RoPE applied in place later);
            # v natural [T, F] stored as [128, t_tiles, F].
            qk_t = [
                persist.tile([D, T], BF, name=f"qk_t{ft}", tag=f"qk_t{ft}")
                for ft in range(2 * hpc)
            ]
            v_sb = persist.tile([128, cfg.t_tiles, F], BF, name="v_sb")

            # ------------- Phase A: fused QKV projection + RoPE -------------
            with (
                tc.tile_pool(name="xpool", bufs=3) as xpool,
                tc.tile_pool(name="rope_tmp", bufs=2) as rope_tmp,
                tc.tile_pool(name="qkv_ps", bufs=4, space="PSUM") as qkv_ps,
            ):

                def load_x_chunk(tci, split=False):
                    x_t = xpool.tile([128, nct, 512], BF, tag="x")
                    lo = xT[:, tci * 512 : (tci + 1) * 512]
                    src = lo.rearrange("(c p) t -> p c t", p=128)
                    if split:
                        # quarters: first matmul chain starts pipelining
                        # after 0.5MB; low-cc quarters on the SP ring
                        q = nct // 4
                        for i in range(4):
                            eng = nc.sync if i < 2 else nc.scalar
                            eng.dma_start(
                                out=x_t[:, i * q : (i + 1) * q, :],
                                in_=src[:, i * q : (i + 1) * q, :],
                            )
                    else:
                        nc.sync.dma_start(out=x_t, in_=src)
                    return x_t

                # wqk ft0 FIRST on the ACT ring (it gates the very first
                # matmul chain together with x chunk 0's sync half)
                nc.scalar.dma_start(out=wqkf_sb[0], in_=wqk[0:128, :])
                x_chunks = [None] * cfg.t_chunks
                x_chunks[0] = load_x_chunk(0, split=True)
                for ft in range(1, 2 * hpc):
                    nc.scalar.dma_start(
                        out=wqkf_sb[ft], in_=wqk[ft * 128 : (ft + 1) * 128, :]
                    )
                if cfg.t_chunks > 1:
                    x_chunks[1] = load_x_chunk(1)
                nc.sync.dma_start(
                    out=wv_sb, in_=wv.rearrange("(c p) f -> p c f", p=128)
                )
                cossin_sb = consts.tile([D, 2 * T], BF, name="cossin_sb")
                nc.scalar.dma_start(out=cossin_sb, in_=cossinT[:, :])
                cos_sb = cossin_sb[:, 0:T]
                sin_sb = cossin_sb[:, T : 2 * T]
                kc_sb = consts.tile([128, 384], BF, name="kc_sb")
                nc.scalar.dma_start(out=kc_sb, in_=kconsts[:, :])
                ones_sb = kc_sb[:, 0:128]
                ident_sb = kc_sb[:, 128:256]
                maskT_sb = kc_sb[:, 256:384]

                def rope_inplace(ft):
                    t_cos = rope_tmp.tile([D, T], BF, tag="t_cos")
                    nc.vector.tensor_mul(t_cos, qk_t[ft], cos_sb)
                    t_shift = rope_tmp.tile([D, T], BF, tag="t_shift")
                    nc.vector.tensor_copy(t_shift[0:64, :], qk_t[ft][64:128, :])
                    nc.vector.tensor_copy(t_shift[64:128, :], qk_t[ft][0:64, :])
                    nc.vector.tensor_mul(t_shift, t_shift, sin_sb)
                    nc.vector.tensor_add(qk_t[ft], t_cos, t_shift)

                for tci in range(cfg.t_chunks):
                    x_ch = x_chunks[tci]
                    for ft in range(2 * hpc):
                        ps_qk = qkv_ps.tile([128, 512], FP, tag="ps_qk")
                        for cc in range(nct):
                            nc.tensor.matmul(
                                ps_qk,
                                lhsT=wqkf(ft)[:, cc * 128 : (cc + 1) * 128],
                                rhs=x_ch[:, cc, :],
                                start=(cc == 0),
                                stop=(cc == nct - 1),
                            )
                        nc.vector.tensor_copy(
                            qk_t[ft][:, tci * 512 : (tci + 1) * 512], ps_qk
                        )
                        if tci == cfg.t_chunks - 1:
                            rope_inplace(ft)
                    for tt in range(4):
                        ps_v = qkv_ps.tile([128, F], FP, tag="ps_v")
                        for cc in range(nct):
                            nc.tensor.matmul(
                                ps_v,
                                lhsT=x_ch[:, cc, tt * 128 : (tt + 1) * 128],
                                rhs=wv_sb[:, cc, :],
                                start=(cc == 0),
                                stop=(cc == nct - 1),
                            )
                        nc.scalar.copy(v_sb[:, tci * 4 + tt, :], ps_v)
                    if tci + 2 < cfg.t_chunks:
                        x_chunks[tci + 2] = load_x_chunk(tci + 2)

            # wout loads now so it lands during phase B
            wout_sb = wo_pool.tile([128, hpc, C], BF, name="wout_sb")
            nc.scalar.dma_start(
                out=wout_sb, in_=wout.rearrange("(h p) c -> p h c", p=128)
            )

            # ---------------- Phase B: causal attention ----------------
            # One flattened, software-pipelined stream of key-tile pairs.
            otn = [[None] * cfg.t_chunks for _ in range(hpc)]
            with (
                tc.tile_pool(name="expp", bufs=6) as expp,
                tc.tile_pool(name="foldp", bufs=6) as foldp,
                tc.tile_pool(name="rsp", bufs=3) as rsp,
                tc.tile_pool(name="sc_ps", bufs=2, space="PSUM") as sc_ps,
                tc.tile_pool(name="av_ps", bufs=2, space="PSUM") as av_ps,
                tc.tile_pool(name="rs_ps", bufs=2, space="PSUM") as rs_ps,
            ):

                def pair_stream():
                    for h in range(hpc):
                        for qc in range(cfg.t_chunks):
                            nkt = (qc + 1) * 4
                            ctx = {
                                "h": h,
                                "qc": qc,
                                "nkt": nkt,
                                "nkp": nkt // 2,
                                "q_sl": qk_t[h][:, qc * 512 : (qc + 1) * 512],
                                "k_h": qk_t[hpc + h],
                                "ps_av": None,
                                "ps_rs": None,
                                "rs_started": False,
                                "pending_folds": [],
                            }
                            for jp in range(ctx["nkp"]):
                                yield ctx, jp

                def rs_mm(ctx, s, rhs_ap, last):
                    nc.tensor.matmul(
                        ctx["ps_rs"][:, s:512],
                        lhsT=ones_sb,
                        rhs=rhs_ap,
                        start=not ctx["rs_started"],
                        stop=last,
                    )
                    ctx["rs_started"] = True

                def issue_scores(ctx, jp):
                    """Scores + additive mask + exp + gpsimd fold for pair jp."""
                    nkt = ctx["nkt"]
                    j0, j1 = 2 * jp, 2 * jp + 1
                    sd0 = (j0 - (nkt - 4)) * 128
                    sd1 = (j1 - (nkt - 4)) * 128
                    d0, d1 = sd0 >= 0, sd1 >= 0
                    s0, s1 = max(0, sd0), max(0, sd1)
                    ps_sc = sc_ps.tile([128, 1024], FP, tag="ps_sc")
                    expT = expp.tile([128, 1024], BF, tag="expT")
                    for half, j, s, dg in ((0, j0, s0, d0), (1, j1, s1, d1)):
                        base = half * 512
                        nc.tensor.matmul(
                            ps_sc[:, base + s : base + 512],
                            lhsT=ctx["k_h"][:, j * 128 : (j + 1) * 128],
                            rhs=ctx["q_sl"][:, s:512],
                            start=True,
                            stop=not dg,
                        )
                        if dg:
                            # additive causal mask on the 128-col triangular
                            # block; closes the accumulation group.
                            nc.tensor.matmul(
                                ps_sc[:, base + s : base + s + 128],
                                lhsT=maskT_sb,
                                rhs=ident_sb,
                                start=False,
                                stop=True,
                            )
                    if s0 == 0 and s1 == 0:
                        nc.scalar.activation(
                            expT, ps_sc, Exp, scale=float(cfg.scale)
                        )
                    else:
                        for half, s in ((0, s0), (1, s1)):
                            base = half * 512
                            nc.scalar.activation(
                                expT[:, base + s : base + 512],
                                ps_sc[:, base + s : base + 512],
                                Exp,
                                scale=float(cfg.scale),
                            )
                    fold = None
                    if not (d0 or d1):
                        fold = foldp.tile([128, 512], BF, tag="fold")
                        nc.gpsimd.tensor_add(
                            fold, expT[:, 0:512], expT[:, 512:1024]
                        )
                    return (jp, j0, j1, s0, s1, d0 or d1, expT, fold)

                def issue_consume(ctx, st):
                    """attn@v + rowsum matmuls for a previously issued pair
                    (one-pair lag keeps TensorE fed); finalizes the (h, qc)
                    iteration after its last pair."""
                    jp, j0, j1, s0, s1, diag, expT, fold = st
                    h, nkp = ctx["h"], ctx["nkp"]
                    if jp == 0:
                        ctx["ps_av"] = av_ps.tile(
                            [128, 512], FP, tag="ps_av", name="ps_av"
                        )
                        ctx["ps_rs"] = rs_ps.tile(
                            [128, 512], FP, tag="ps_rs", name="ps_rs"
                        )
                    first, last = (jp == 0), (jp == nkp - 1)
                    # drain age>=2 folds BEFORE attn@v: their inputs are
                    # guaranteed ready, absorbing jitter while this pair's
                    # exp finishes on ScalarE
                    pend = ctx["pending_folds"]
                    while len(pend) >= 2:
                        rs_mm(ctx, 0, pend.pop(0), False)
                    nc.tensor.matmul(
                        ctx["ps_av"][:, s0:512],
                        lhsT=v_sb[:, j0, h * 128 : (h + 1) * 128],
                        rhs=expT[:, s0 : 512],
                        start=first,
                        stop=False,
                    )
                    nc.tensor.matmul(
                        ctx["ps_av"][:, s1:512],
                        lhsT=v_sb[:, j1, h * 128 : (h + 1) * 128],
                        rhs=expT[:, 512 + s1 : 1024],
                        start=False,
                        stop=last,
                    )
                    # remaining rowsum matmuls (fold lag keeps the gpsimd
                    # add ~2 pairs ahead of its in-order rowsum matmul)
                    if diag:
                        while pend:
                            rs_mm(ctx, 0, pend.pop(0), False)
                        rs_mm(ctx, s0, expT[:, s0:512], False)
                        rs_mm(ctx, s1, expT[:, 512 + s1 : 1024], last)
                    else:
                        pend.append(fold)
                    if last:
                        # 1/rowsum (bf16: ~0.4% uniform per-query scale, well
                        # within budget) + normalize on DVE
                        qc = ctx["qc"]
                        rsrec = rsp.tile([128, 512], BF, tag="rsrec")
                        with nc.allow_low_precision(
                            reason="softmax denom recip in bf16"
                        ):
                            nc.vector.reciprocal(out=rsrec, in_=ctx["ps_rs"])
                        o = otp.tile(
                            [128, 512], BF, name=f"otn{h}_{qc}",
                            tag=f"otn{h}_{qc}",
                        )
                        nc.vector.tensor_mul(o, ctx["ps_av"], rsrec)
                        otn[h][qc] = o

                prev = None
                for ctx_jp in pair_stream():
                    st = issue_scores(*ctx_jp)
                    if prev is not None:
                        issue_consume(prev[0], prev[1])
                    prev = (ctx_jp[0], st)
                issue_consume(prev[0], prev[1])

            # ---------------- Phase D: out-projection ----------------
            # bf16 partial out; PSUM->SBUF copies alternate ScalarE/DVE;
            # one DMA per row-tile.
            with (
                tc.tile_pool(name="osb_pool", bufs=2) as osb_pool,
                tc.tile_pool(name="o_ps", bufs=4, space="PSUM") as o_ps,
            ):
                n_groups = cfg.t_tiles // 4
                for g in range(n_groups):
                    # last group: DMA per 512-chunk right after its copy so
                    # the tail drains as early as possible (extra triggers
                    # land on an idle ring at the end)
                    fine = g == n_groups - 1
                    o_sb = osb_pool.tile([128, 4, C], BF, tag="o_sb")
                    for ti in range(4):
                        tt = g * 4 + ti
                        qc, off = tt // 4, (tt % 4) * 128
                        for n in range(cfg.n_chunks):
                            ps_o = o_ps.tile([128, 512], FP, tag="ps_o")
                            for h in range(hpc):
                                nc.tensor.matmul(
                                    ps_o,
                                    lhsT=otn[h][qc][:, off : off + 128],
                                    rhs=wout_sb[:, h, n * 512 : (n + 1) * 512],
                                    start=(h == 0),
                                    stop=(h == hpc - 1),
                                )
                            o_chunk = o_sb[:, ti, n * 512 : (n + 1) * 512]
                            if (tt * cfg.n_chunks + n) % 2 == 0:
                                nc.scalar.copy(o_chunk, ps_o)
                            else:
                                nc.vector.tensor_copy(o_chunk, ps_o)
                            if fine:
                                nc.sync.dma_start(
                                    out=out[
                                        tt * 128 : (tt + 1) * 128,
                                        n * 512 : (n + 1) * 512,
                                    ],
                                    in_=o_chunk,
                                )
                        if not fine:
                            nc.sync.dma_start(
                                out=out[tt * 128 : (tt + 1) * 128, :],
                                in_=o_sb[:, ti, :],
                            )

    return nc


def rope_tables(T, dtype=np.float32):
    inv_freq = 1.0 / (ROPE_THETA ** (np.arange(0, D, 2, dtype=np.float32) / D))
    t = np.arange(T, dtype=np.float32)
    freqs = np.outer(t, inv_freq)  # [T, D/2]
    emb = np.concatenate([freqs, freqs], axis=-1)  # [T, D]
    return np.cos(emb).astype(dtype), np.sin(emb).astype(dtype)


def make_core_inputs(cfg: Cfg, x_b, w_qkv, w_out, cos, sin, hg):
    """Per-core input dict. x_b [T, C] fp32; w_qkv [C, 3C']; w_out [C', C];
    cos/sin [T, D]; hg = head-group index within the batch group."""
    T, C, hpc = cfg.T, cfg.C, cfg.hpc
    F = hpc * D
    H = w_qkv.shape[1] // 3 // D  # total heads in this (possibly shrunk) problem
    CQ = H * D

    f0 = hg * F
    xT = np.ascontiguousarray(x_b.T).astype(BF_NP)
    wq = w_qkv[:, f0 : f0 + F]
    wk = w_qkv[:, CQ + f0 : CQ + f0 + F]
    W = np.concatenate([wq, wk], axis=1)  # [C, 2F]
    # pack per-ft: wqk[ft*128+p, cc*128+f] = W[cc*128+p, ft*128+f]
    nft, ncc = 2 * hpc, C // 128
    wqk = np.ascontiguousarray(
        W.reshape(ncc, 128, nft, 128).transpose(2, 1, 0, 3).reshape(
            nft * 128, ncc * 128
        )
    ).astype(BF_NP)
    wv = np.ascontiguousarray(w_qkv[:, 2 * CQ + f0 : 2 * CQ + f0 + F]).astype(BF_NP)
    wout = np.ascontiguousarray(w_out[f0 : f0 + F, :]).astype(BF_NP)

    cosT = np.ascontiguousarray(cos.T).astype(np.float32)  # [D, T]
    sinT = np.ascontiguousarray(sin.T).astype(np.float32)
    sinT[0:64, :] *= -1.0  # bake rotate_half sign
    cossinT = np.concatenate([cosT, sinT], axis=1).astype(BF_NP)

    # packed small consts [128, 384]: all-ones | identity | maskT where
    # maskT[q, k] = MASK_NEG iff k > q (transposed for lhsT use w/ rhs=ident)
    q_idx = np.arange(128)[:, None]
    k_idx = np.arange(128)[None, :]
    maskT = np.where(k_idx > q_idx, np.float32(MASK_NEG), np.float32(0.0))
    kconsts = np.concatenate(
        [np.ones((128, 128), np.float32), np.eye(128, dtype=np.float32), maskT],
        axis=1,
    ).astype(BF_NP)

    return {
        "xT": xT,
        "wqk": wqk,
        "wv": wv,
        "wout": wout,
        "cossinT": np.ascontiguousarray(cossinT),
        "kconsts": np.ascontiguousarray(kconsts),
    }


_NC_CACHE = {}


def _get_nc(cfg: Cfg):
    key = (cfg.T, cfg.C, cfg.hpc)
    if key not in _NC_CACHE:
        nc = build_attention(cfg)
        _split_multi_waits(nc)  # HW codegen needs ≤1 wait per instruction
        _NC_CACHE[key] = nc
    return _NC_CACHE[key]


def kernel(x, cos, sin, w_qkv, w_out, trace=False, tmpdir=None):
    """Full-problem entry point: full inputs in, full [B, T, C] output back."""
    from concourse.bass_utils import run_bass_kernel_spmd

    x = np.asarray(x, dtype=np.float32)
    cos = np.asarray(cos, dtype=np.float32)
    sin = np.asarray(sin, dtype=np.float32)
    w_qkv = np.asarray(w_qkv, dtype=np.float32)
    w_out = np.asarray(w_out, dtype=np.float32)

    cfg = Cfg()
    nc = _get_nc(cfg)

    in_maps = []
    for c in range(N_CORES):
        b, hg = c // 4, c % 4
        in_maps.append(
            make_core_inputs(cfg, x[b], w_qkv, w_out, cos, sin, hg)
        )

    res = run_bass_kernel_spmd(
        nc,
        in_maps,
        core_ids=list(range(N_CORES)),
        trace=trace,
        tmpdir=tmpdir,
    )
    partials = [np.asarray(r["out"], dtype=np.float32) for r in res.results]
    out = np.empty((B, cfg.T, cfg.C), dtype=np.float32)
    for b in range(B):
        out[b] = partials[4 * b] + partials[4 * b + 1]
        out[b] += partials[4 * b + 2]
        out[b] += partials[4 * b + 3]
    if trace:
        return out, res
    return out
